# revision 47
# baseline (speedup 1.0000x reference)
"""Block-causal GQA attention on 8 trn2 NeuronCores.

Sharding: core = b*4 + g  (b in {0,1} batch, g in {0..3} kv-head group).
Each core computes, for its batch b and kv group g (4 q-heads, 1 kv head):
    partial_out = softmax_blockcausal(rope(x@Wq_g) @ rope(x@Wk_g)^T) @ (x@Wv_g) @ Wo_g
Host sums the 4 group partials per batch.

Device design (bf16 matmuls, f32 PSUM):
  - Host passes x^T, so Q^T/K^T/V^T come out of projections with d on
    partitions and no on-device transposes; RoPE (sign folded into the sin
    table) happens on DVE during PSUM eviction.  V^T is DMA-xbar-transposed
    into V_aug = [V | ones].
  - Projections run c-chunk-outer in PSUM waves (K+V, Q0+Q1, Q2+Q3) so PE
    work starts as soon as the first x^T chunk lands.
  - Attention per (head, 1024-wide tq half): S^T[tk,tq] = K^T.T @ Q^T,
    exp on ACT (scale=1/sqrt(128); scores are O(1) so no max subtraction),
    then per 128-wide tq tile: [Y|Z][tq,129] += P^T_tile.T @ V_aug
    (P^T stationary, fused softmax denominator in column 128).
    Normalize: rz = 1/Z [tq,1], Y *= rz via per-partition tensor_scalar,
    DMA-transpose Y tile into Y^T[d, tq].
  - O[t,n] = sum_h Y_h^T.T @ Wo_h accumulated in PSUM over heads;
    evictions alternate DVE/ACT.
"""
import os
import sys
import numpy as np

for _p in ("/opt/trn_rl_repo",):
    if _p not in sys.path and os.path.isdir(_p):
        sys.path.insert(0, _p)

import ml_dtypes

BF16 = ml_dtypes.bfloat16

B = 2
T = 2048
C = 2048
HD = 128
NHL = 4           # q heads per core
NT = T // 128     # 16 query/key tiles
NCH = C // 128    # 16 contraction chunks
HW = T // 2       # tq half width
SCALE = 1.0 / float(np.sqrt(np.float32(HD)))

_CACHE = {}


def _build_nc():
    import concourse.bass as bass
    import concourse.mybir as mybir
    import concourse.tile as tile
    from concourse import bacc

    dt = mybir.dt
    f32 = dt.float32
    bf = dt.bfloat16
    Exp = mybir.ActivationFunctionType.Exp

    nc = bacc.Bacc(None, target_bir_lowering=False)

    # weights host-prelaid as [partition, chunk, m] so each DMA is 128 fat
    # contiguous descriptors instead of 2048 small ones
    xT = nc.declare_dram_parameter("xT", [C, T], bf, isOutput=False)
    wq = nc.declare_dram_parameter("wq", [128, NCH, NHL * HD], bf, isOutput=False)
    wk = nc.declare_dram_parameter("wk", [128, NCH, HD], bf, isOutput=False)
    wv = nc.declare_dram_parameter("wv", [128, NCH, HD], bf, isOutput=False)
    wo = nc.declare_dram_parameter("wo", [128, NHL, C], bf, isOutput=False)
    cosT = nc.declare_dram_parameter("cosT", [HD, T], bf, isOutput=False)
    sinT = nc.declare_dram_parameter("sinT", [HD, T], bf, isOutput=False)
    o = nc.declare_dram_parameter("o_part", [T, C], f32, isOutput=True)

    with tile.TileContext(nc) as tc:
        with tc.tile_pool(name="consts", bufs=1) as consts:
            # ---- static SBUF loads (order = DMA priority) ----
            wk_sb = consts.tile([128, NCH, HD], bf, name="wk_sb")
            nc.sync.dma_start(wk_sb, wk[:, :, :])
            wv_sb = consts.tile([128, NCH, HD], bf, name="wv_sb")
            nc.sync.dma_start(wv_sb, wv[:, :, :])

            cos_sb = consts.tile([128, T], bf, name="cos_sb")
            sin_sb = consts.tile([128, T], bf, name="sin_sb")
            wq_sb = consts.tile([128, NCH, NHL * HD], bf, name="wq_sb")
            wo_sb = consts.tile([128, NHL, C], bf, name="wo_sb")

            # V_aug = [V | ones]: col 128 preset to 1, cols 0:128 filled by
            # DMA-transpose from V^T after the V projection.  Rows are 256
            # wide so each tile's dst offset stays 512B-aligned — the DMA
            # xbar transpose corrupts data at unaligned dst offsets.
            vaug_sb = consts.tile([128, NT, 2 * HD], bf, name="vaug_sb")
            nc.vector.memset(vaug_sb[:, :, HD:HD + 1], 1.0)

            # warm the ACT exp table set during phase 1
            dumm = consts.tile([1, 8], f32, name="dumm")
            nc.vector.memset(dumm, 0.0)
            nc.scalar.activation(dumm, dumm, Exp)

            # persistent activations
            kt_sb = consts.tile([128, T], bf, name="kt_sb")
            vt_sb = consts.tile([128, T], bf, name="vt_sb")
            qt_sb = [consts.tile([128, T], bf, name=f"qt{h}") for h in range(NHL)]
            yt_sb = [consts.tile([128, T], bf, name=f"yt{h}") for h in range(NHL)]

            # ============ phase 1: projections (c-outer waves) ============
            with tc.tile_pool(name="xtp", bufs=1) as xtp, \
                 tc.tile_pool(name="proj", bufs=1) as proj, \
                 tc.tile_pool(name="proj_psum", bufs=2, space="PSUM") as pp:

                xt_r = xT.rearrange("(n p) t -> n p t", p=128)
                xt_sb = []
                for cch in range(NCH):
                    xt_c = xtp.tile([128, T], bf, name=f"xt{cch}")
                    nc.sync.dma_start(xt_c, xt_r[cch])
                    xt_sb.append(xt_c)
                    if cch == 10:
                        # wq arrives just before the Q waves need it
                        nc.sync.dma_start(wq_sb, wq[:, :, :])
                nc.sync.dma_start(cos_sb, cosT[:, :])
                nc.sync.dma_start(sin_sb, sinT[:, :])
                nc.sync.dma_start(wo_sb, wo[:, :, :])

                def rope_evict(ps, jsl, dst):
                    # dst[:, jsl] = ps * cos + rot_half(ps) * sin  (bf16).
                    # ACT does the PSUM eviction; DVE runs at bf16 2x.
                    t0 = proj.tile([128, 512], bf, tag="t0", bufs=4)
                    t1 = proj.tile([128, 512], bf, tag="t1", bufs=4)
                    t2 = proj.tile([128, 512], bf, tag="t2", bufs=4)
                    # sin table halves are pre-swapped on host so each mul
                    # reads both SBUF inputs at the same base partition
                    # (walrus requires equal SBUF base partitions).
                    nc.scalar.copy(t0, ps)
                    nc.vector.tensor_mul(t1, t0, cos_sb[:, jsl])
                    nc.vector.tensor_mul(t2[0:64], t0[64:128], sin_sb[64:128, jsl])
                    nc.vector.tensor_mul(t2[64:128], t0[0:64], sin_sb[0:64, jsl])
                    nc.vector.tensor_add(dst[:, jsl], t1, t2)

                # warm the PE clock (HAM) with throwaway matmuls while the
                # input DMAs stream in; results are never read
                warm_ps = pp.tile([128, 512], f32, tag="pj", bufs=8,
                                  name="warm_ps")
                for _ in range(28):
                    nc.tensor.matmul(warm_ps[0:1, :],
                                     vaug_sb[:, 0, HD:HD + 1],
                                     kt_sb[:, 0:512], start=True, stop=True)

                # -- wave 1: K and V (c-outer so PE starts with first chunk) --
                ps_k = [pp.tile([128, 512], f32, tag="pj", bufs=8,
                                name=f"ps_k{j}") for j in range(4)]
                ps_v = [pp.tile([128, 512], f32, tag="pj", bufs=8,
                                name=f"ps_v{j}") for j in range(4)]
                for cch in range(NCH):
                    st, sp = (cch == 0), (cch == NCH - 1)
                    for j in range(T // 512):
                        jsl = slice(512 * j, 512 * (j + 1))
                        nc.tensor.matmul(ps_k[j], wk_sb[:, cch, :],
                                         xt_sb[cch][:, jsl], start=st, stop=sp)
                        nc.tensor.matmul(ps_v[j], wv_sb[:, cch, :],
                                         xt_sb[cch][:, jsl], start=st, stop=sp)
                for j in range(T // 512):
                    jsl = slice(512 * j, 512 * (j + 1))
                    rope_evict(ps_k[j], jsl, kt_sb)
                    nc.scalar.copy(vt_sb[:, jsl], ps_v[j])
                for i in range(NT):
                    nc.sync.dma_start_transpose(
                        vaug_sb[:, i, 0:HD], vt_sb[:, 128 * i:128 * (i + 1)]
                    )

                # -- Q: (h, j)-sequential, c-inner; evictions pipeline via
                #    the 8-slot psum rotation --
                for h in range(NHL):
                    hsl = slice(HD * h, HD * (h + 1))
                    for j in range(T // 512):
                        jsl = slice(512 * j, 512 * (j + 1))
                        ps_q = pp.tile([128, 512], f32, tag="pj", bufs=8,
                                       name=f"ps_q{h}_{j}")
                        for cch in range(NCH):
                            nc.tensor.matmul(
                                ps_q, wq_sb[:, cch, hsl], xt_sb[cch][:, jsl],
                                start=(cch == 0), stop=(cch == NCH - 1))
                        rope_evict(ps_q, jsl, qt_sb[h])

            # ============ phases 2+3: attention + output projection ======
            # One PSUM pool: tag "s" (2x2 banks) + tag "b1" (4x1 bank shared
            # by the [Y|Z] accumulators and the O-proj tiles) = 8 banks.
            with tc.tile_pool(name="attn", bufs=1) as ap, \
                 tc.tile_pool(name="attn_psum", bufs=1, space="PSUM") as apsum:

                def y_group(h, half, il, tiles):
                    """One [Y|Z] accumulation + normalize + transpose-out."""
                    gi = (HW // 128) * half + il
                    ps_yz = apsum.tile([128, 512], f32, tag="b1", bufs=2,
                                       name="ps_yz")
                    for tk in range(gi + 1):
                        nc.tensor.matmul(
                            ps_yz[:, 0:HD + 1],
                            tiles[tk][:, 128 * il:128 * (il + 1)],
                            vaug_sb[:, tk, 0:HD + 1],
                            start=(tk == 0), stop=(tk == gi))
                    rz = ap.tile([128, 1], f32, tag="rz", bufs=8)
                    nc.vector.reciprocal(rz, ps_yz[:, HD:HD + 1])
                    ysb = ap.tile([128, HD], bf, tag="ysb", bufs=8)
                    nc.vector.tensor_scalar_mul(ysb, ps_yz[:, 0:HD], rz)
                    nc.sync.dma_start_transpose(
                        yt_sb[h][:, 128 * gi:128 * (gi + 1)], ysb)

                def attn_block(h, half, prev):
                    """S^T + exp for this tq half, with the previous block's
                    Y-groups woven between the tk iterations so PE always
                    has queued work while ACT catches up on exps."""
                    tq0 = HW * half
                    ntk = (tq0 + HW) // 128
                    tiles = []
                    pi = 0

                    def emit_y(n):
                        nonlocal pi
                        while n > 0 and prev is not None and pi < HW // 128:
                            y_group(prev[0], prev[1], pi, prev[2])
                            pi += 1
                            n -= 1

                    for tk in range(ntk):
                        lo = max(0, 128 * tk - tq0)
                        ps_s = apsum.tile([128, HW], f32, tag="s", bufs=3)
                        chunks = ([(lo, 512), (512, HW)] if lo < 512
                                  else [(lo, HW)])
                        for (a, bnd) in chunks:
                            nc.tensor.matmul(
                                ps_s[:, a:bnd],
                                kt_sb[:, 128 * tk:128 * (tk + 1)],
                                qt_sb[h][:, tq0 + a:tq0 + bnd],
                                start=True, stop=True)
                        p_t = ap.tile([128, HW], bf, tag="p", bufs=34)
                        nc.scalar.activation(p_t[:, lo:HW], ps_s[:, lo:HW],
                                             Exp, scale=SCALE)
                        tiles.append(p_t)
                        if tk >= 1 and (ntk == 8 or tk % 2 == 1):
                            emit_y(1)
                    emit_y(HW // 128)
                    return tiles

                def oproj(t0, t1, dve_only):
                    for ti in range(t0, t1):
                        tsl = slice(128 * ti, 128 * (ti + 1))
                        for n in range(C // 512):
                            nsl = slice(512 * n, 512 * (n + 1))
                            ps_o = apsum.tile([128, 512], f32, tag="b1",
                                              bufs=2, name="ps_o")
                            for h in range(NHL):
                                nc.tensor.matmul(
                                    ps_o, yt_sb[h][:, tsl], wo_sb[:, h, nsl],
                                    start=(h == 0), stop=(h == NHL - 1))
                            ob = ap.tile([128, 512], f32, tag="ob", bufs=8)
                            if dve_only or (ti * 4 + n) % 2 == 0:
                                nc.vector.tensor_copy(ob, ps_o)
                            else:
                                nc.scalar.copy(ob, ps_o)
                            nc.gpsimd.dma_start(o[tsl, nsl], ob)

                # half-outer: all heads' half-0 Y tiles finish first, then
                # O-proj row tiles 0..7 drip into the half-1 attention
                # windows (which are ACT/exp-bound) to keep PE fed.
                prev = None
                od = 0
                for half in range(2):
                    for h in range(NHL):
                        cur = (h, half, attn_block(h, half, prev))
                        if prev is not None:
                            if prev[1] == 0 and prev[0] == NHL - 1:
                                od = 0
                                oproj(od, od + 2, dve_only=True)
                                od += 2
                            elif prev[1] == 1 and od > 0:
                                oproj(od, od + 2, dve_only=True)
                                od += 2
                        prev = cur
                for il in range(HW // 128):
                    y_group(prev[0], prev[1], il, prev[2])
                oproj(od, NT, dve_only=False)

    nc.finalize()
    return nc


def _tables():
    freqs = 1.0 / (10000.0 ** (np.arange(0, HD, 2, dtype=np.float32) / HD))
    t = np.arange(T, dtype=np.float32)
    emb = np.outer(t, freqs)                  # [T, 64]
    cos_t = np.cos(emb).T.astype(np.float32)  # [64, T]
    sin_t = np.sin(emb).T.astype(np.float32)
    cosT = np.ascontiguousarray(np.concatenate([cos_t, cos_t], 0)).astype(BF16)
    # halves swapped: row d holds the factor multiplying t0[(d+64)%128]
    # when writing t2[d ^ 64 half]; see rope_evict
    sinT = np.ascontiguousarray(np.concatenate([sin_t, -sin_t], 0)).astype(BF16)
    return cosT, sinT


def _get_nc():
    if "nc" not in _CACHE:
        _CACHE["nc"] = _build_nc()
    return _CACHE["nc"]


def kernel(x, Wq, Wk, Wv, Wo, _trace=False):
    from concourse.bass_utils import run_bass_kernel_spmd

    x = np.asarray(x, dtype=np.float32)
    cosT, sinT = _tables()

    def chunked(w):
        # [K, m] -> [128, K//128, m] (partition-major, contiguous)
        k, m = w.shape
        return np.ascontiguousarray(
            w.reshape(k // 128, 128, m).transpose(1, 0, 2)).astype(BF16)

    in_maps = []
    for core in range(8):
        b, g = divmod(core, 4)
        in_maps.append({
            "xT": np.ascontiguousarray(x[b].T).astype(BF16),
            "wq": chunked(Wq[:, 512 * g:512 * (g + 1)]),
            "wk": chunked(Wk[:, 128 * g:128 * (g + 1)]),
            "wv": chunked(Wv[:, 128 * g:128 * (g + 1)]),
            "wo": chunked(Wo[512 * g:512 * (g + 1), :]),
            "cosT": cosT,
            "sinT": sinT,
        })

    nc = _get_nc()
    res = run_bass_kernel_spmd(nc, in_maps, list(range(8)), trace=_trace)
    parts = [res.results[c]["o_part"] for c in range(8)]
    out = np.empty((B, T, C), dtype=np.float32)
    for b in range(B):
        out[b] = parts[4 * b] + parts[4 * b + 1] + parts[4 * b + 2] + parts[4 * b + 3]
    if _trace:
        return out, res
    return out
